# revision 6
# baseline (speedup 1.0000x reference)
"""Bass/Trainium2 kernel for Kimi-style MLA attention (nn_KimiMLAAttention).

Strategy (8 NeuronCores, tensor-parallel over heads):
  - 16 heads -> 2 heads per core. Each core computes q-projection for its 2
    heads, the (replicated) compressed-kv projection + rmsnorm, per-head
    k-embed / v-unembed from the shared latent, causal attention in a
    TRANSPOSED score layout (scores^T[s, l]), and a partial o_proj against
    its 2-head slice of Wo. Host sums the 8 partial outputs.

Perf design (v2):
  - All SBUF operands bf16 (PE streams bf16 at 1 col/cycle like fp32r, but
    DMA/SBUF/vector costs halve); PSUM accumulation stays fp32.
  - Everything is pipelined per 512-column l-chunk: P0 projections ->
    rmsnorm -> P2 k/v embed -> P3 attention (2 heads) -> P4 partial o_proj,
    with the next chunk's projection matmuls issued between dependency
    stalls so the tensor engine never idles (PE clock ramps to 2.4 GHz only
    after ~3us of continuous execution; gaps drop it to 1.2 GHz).
  - P0 runs k-innermost into 2 rotating PSUM banks (no 8-bank barrier).
  - Causal masking is done ON the PE: a -BIG * lower-triangle matmul is
    accumulated into the score PSUM tile before exp, so the scalar exp
    output needs no vector-side mask multiply. Diagonal-band tiles stream
    only their valid column suffix (~half the band work skipped).
  - Normalizations (rmsnorm rsqrt, softmax 1/colsum) avoid the slow DVE
    InstReciprocal: row = Ln(sum) on scalar -> rank-1 broadcast matmul with
    a -1 (or -0.5) row -> Exp on scalar gives exp(-ln x) = 1/x (or x^-1/2)
    broadcast across partitions.
  - Score matmuls are issued one s-tile ahead of the exp-dependent
    colsum/AV matmuls (software pipelining, PSUM tag ring buffers).
"""

from contextlib import ExitStack

import numpy as np
import ml_dtypes

import concourse.bass as bass
import concourse.tile as tile
from concourse import mybir
from concourse.bass import ds, ts
from concourse.bass_utils import run_bass_kernel_spmd

F32 = mybir.dt.float32
F32R = mybir.dt.float32r
BF = mybir.dt.bfloat16
AF = mybir.ActivationFunctionType


def _patch_tile_tail_drain():
    """walrus's CoreV3 codegen rejects the TileContext tail drain when it
    carries >1 sem waits ("Too many sync wait commands"). Split the waits
    across multiple single-wait drain instructions on the sync engine."""
    if getattr(tile.TileContext, "_tail_drain_patched", False):
        return
    from concourse.vector_clock import ScopedClock

    def _drain_and_barrier(self, tick_clock, wait_clock):
        nc = self.nc
        drain_inst = nc.sync.drain()
        wait_clock.add_sem_waits(
            drain_inst.ins, ScopedClock({None: tick_clock.global_clock})
        )
        inst = drain_inst.ins
        si = inst.sync_info
        if si is not None and si.on_wait is not None and len(si.on_wait) > 1:
            waits = list(si.on_wait)
            upd = list(si.on_update) if si.on_update else []
            inst.sync_info = mybir.SyncInfo(on_wait=waits[:1], on_update=[])
            for i, w in enumerate(waits[1:]):
                extra = nc.sync.drain()
                last = i == len(waits) - 2
                extra.ins.sync_info = mybir.SyncInfo(
                    on_wait=[w], on_update=upd if last else []
                )
        nc.all_engine_barrier()
        assert self.sems is not None
        popped = nc._tile_sem_poison_stack.pop()
        assert popped is self._sem_poison
        nc.clear_and_free_semaphores(list(self.sems.allocated().values()))
        nc.all_engine_barrier()

    tile.TileContext._drain_and_barrier = _drain_and_barrier
    tile.TileContext._tail_drain_patched = True


_patch_tile_tail_drain()


def _split_excess_waits(nc, max_waits=1):
    """walrus's per-instruction sync-wait slots are tiny on this compiler
    build; hoist excess sem waits onto same-engine NoOp carriers placed
    immediately before the instruction (waits fire earlier in the same
    engine stream, so ordering semantics are preserved)."""
    for f in nc.m.functions:
        for bb in f.blocks:
            insts = bb.instructions
            if not any(
                i.sync_info is not None
                and i.sync_info.on_wait
                and len(i.sync_info.on_wait) > max_waits
                for i in insts
            ):
                continue
            out = []
            for inst in insts:
                si = inst.sync_info
                if si is not None and si.on_wait and len(si.on_wait) > max_waits:
                    waits = list(si.on_wait)
                    for w in waits[:-max_waits]:
                        nop = mybir.InstNoOp(
                            name=nc.get_next_instruction_name(), ins=[], outs=[]
                        )
                        nop.engine = inst.engine
                        nop.sync_info = mybir.SyncInfo(on_wait=[w], on_update=[])
                        out.append(nop)
                    inst.sync_info = mybir.SyncInfo(
                        on_wait=waits[-max_waits:],
                        on_update=list(si.on_update) if si.on_update else [],
                    )
                out.append(inst)
            bb.instructions = out


B, L, HID = 1, 2048, 2048
H = 16
NOPE, ROPE, VDIM, LORA = 128, 64, 128, 512
QDIM = NOPE + ROPE
EPS = 1e-5
SCALE = QDIM**-0.5
NCORES = 8
HPC = H // NCORES  # 2 heads per core

LCH = 512  # moving-operand chunk (max fp32 N per matmul / PSUM bank)
NJ = L // LCH  # 4 l-chunks
NK = HID // 128  # 16 contraction tiles for projections
NS = L // 128  # 16 s(key)-tiles
NLAT = LORA // 128  # 4 latent partition tiles
WCOLS = 960  # fused projection weight columns
NEGBIG = -1000.0  # pre-exp causal mask bias (NEGBIG*SCALE ~ -72 per unit)

# wqkv column layout (host packs in this order):
#   lat0 lat1 lat2 lat3 (4x128) | kpe (64) | qn0 (128) | qn1 (128) | qr (128)
MC_LAT = [0, 128, 256, 384]
MC_KPE = 512
MC_QN = [576, 704]
MC_QR = 832


def _build_nc():
    nc = bass.Bass()
    xT_d = nc.dram_tensor("xT", [HID, L], BF, kind="ExternalInput")
    wqkv_d = nc.dram_tensor("wqkv", [HID, WCOLS], BF, kind="ExternalInput")
    we_d = nc.dram_tensor("we", [HPC, LORA, NOPE], BF, kind="ExternalInput")
    wu_d = nc.dram_tensor("wu", [LORA, HPC * VDIM], BF, kind="ExternalInput")
    wo_d = nc.dram_tensor("wo", [HPC * VDIM, HID], BF, kind="ExternalInput")
    nbig_d = nc.dram_tensor("nbig", [128, 896], BF, kind="ExternalInput")
    negdiag_d = nc.dram_tensor("negdiag", [128, 128], BF, kind="ExternalInput")
    ones_col_d = nc.dram_tensor("ones_col_d", [128, 1], BF, kind="ExternalInput")
    mhalf_row_d = nc.dram_tensor("mhalf_row_d", [1, 128], F32R, kind="ExternalInput")
    mone_row_d = nc.dram_tensor("mone_row_d", [1, 128], F32R, kind="ExternalInput")
    y_d = nc.dram_tensor("y", [L, HID], BF, kind="ExternalOutput")

    mm = nc.tensor.matmul

    with tile.TileContext(nc) as tc, ExitStack() as ctx:
        persist = ctx.enter_context(tc.tile_pool(name="persist", bufs=1))
        xtp = ctx.enter_context(tc.tile_pool(name="xtp", bufs=2))
        work = ctx.enter_context(tc.tile_pool(name="work", bufs=1))
        pp = ctx.enter_context(tc.tile_pool(name="pp", bufs=1, space="PSUM"))

        # ---- persistent SBUF tiles ----
        qn = [persist.tile([128, L], BF, name=f"qn{h}", tag=f"qn{h}") for h in range(HPC)]
        qr = persist.tile([128, L], BF, name="qr", tag="qr")
        kpe = persist.tile([128, L], BF, name="kpe", tag="kpe")
        latT = [persist.tile([128, L], BF, name=f"latT{i}", tag=f"latT{i}") for i in range(NLAT)]
        kT = [persist.tile([128, L], BF, name=f"kT{h}", tag=f"kT{h}") for h in range(HPC)]
        vsb = persist.tile([128, NS * HPC * VDIM], BF, name="vsb", tag="vsb")
        outT = [persist.tile([128, L], BF, name=f"outT{h}", tag=f"outT{h}") for h in range(HPC)]
        nbig_sb = persist.tile([128, 896], BF, name="nbig_sb", tag="nbig_sb")
        negdiag = persist.tile([128, 128], BF, name="negdiag", tag="negdiag")
        ones_col = persist.tile([128, 1], BF, name="ones_col", tag="ones_col")
        mhalf_row = persist.tile([1, 128], F32R, name="mhalf_row", tag="mhalf_row")
        mone_row = persist.tile([1, 128], F32R, name="mone_row", tag="mone_row")
        w_sb = [persist.tile([128, WCOLS], BF, name=f"w{k}", tag=f"w{k}") for k in range(NK)]
        we_sb = [
            [persist.tile([128, NOPE], BF, name=f"we{h}{i}", tag=f"we{h}{i}") for i in range(NLAT)]
            for h in range(HPC)
        ]
        wu_sb = [persist.tile([128, HPC * VDIM], BF, name=f"wu{i}", tag=f"wu{i}") for i in range(NLAT)]
        wo_sb = [persist.tile([128, HID], BF, name=f"wo{h}", tag=f"wo{h}") for h in range(HPC)]
        eps_col = persist.tile([128, 1], F32, name="eps_col", tag="eps_col")
        nc.vector.memset(eps_col, EPS)

        # ---- startup DMAs ----
        # sync queue: fused-projection weights first (TE consumes k-order),
        # then the small attention constants, then o_proj weights.
        for k in range(NK):
            nc.sync.dma_start(out=w_sb[k], in_=wqkv_d[ts(k, 128), :])
        nc.sync.dma_start(out=ones_col, in_=ones_col_d[:, :])
        nc.sync.dma_start(out=mhalf_row, in_=mhalf_row_d[:, :])
        nc.sync.dma_start(out=mone_row, in_=mone_row_d[:, :])
        nc.sync.dma_start(out=negdiag, in_=negdiag_d[:, :])
        nc.sync.dma_start(out=nbig_sb, in_=nbig_d[:, :])
        for h in range(HPC):
            nc.sync.dma_start(out=wo_sb[h], in_=wo_d[ts(h, 128), :])
        # activation queue: x tiles for chunk 0, then embed/unembed weights.
        xt = [[None] * NK for _ in range(NJ)]

        def load_xt(j):
            for k in range(NK):
                t = xtp.tile([128, LCH], BF, name=f"xt{k}", tag=f"xt{k}")
                nc.scalar.dma_start(out=t, in_=xT_d[ts(k, 128), ds(j * LCH, LCH)])
                xt[j][k] = t

        load_xt(0)
        for h in range(HPC):
            for i in range(NLAT):
                nc.scalar.dma_start(out=we_sb[h][i], in_=we_d[h, ts(i, 128), :])
        for i in range(NLAT):
            nc.scalar.dma_start(out=wu_sb[i], in_=wu_d[ts(i, 128), :])

        # ---- phase bodies ----
        def p0_lat(j):
            """latent+kpe m-chunks for l-chunk j, squares+ssq+Ln of rmsnorm."""
            jc = ds(j * LCH, LCH)
            ssq = pp.tile([1, LCH], F32, name="ssq", tag="rowacc", bufs=2)
            for i in range(NLAT):
                acc = pp.tile([128, LCH], F32, name="acc", tag="accps", bufs=3)
                for k in range(NK):
                    mm(acc, w_sb[k][:, ds(MC_LAT[i], 128)], xt[j][k],
                       start=(k == 0), stop=(k == NK - 1))
                nc.vector.tensor_copy(latT[i][:, jc], acc)
                sq = work.tile([128, LCH], BF, name="sq", tag="sq", bufs=2)
                nc.scalar.activation(sq, acc, AF.Square)
                mm(ssq, ones_col, sq, start=(i == 0), stop=(i == NLAT - 1))
            # kpe chunk (64 cols), duplicated onto partitions 64:128 via DMA
            acc = pp.tile([128, LCH], F32, name="acc", tag="accps", bufs=3)
            for k in range(NK):
                mm(acc[0:64, :], w_sb[k][:, ds(MC_KPE, 64)], xt[j][k],
                   start=(k == 0), stop=(k == NK - 1))
            nc.vector.tensor_copy(kpe[0:64, jc], acc[0:64, :])
            nc.sync.dma_start(out=kpe[64:128, jc], in_=kpe[0:64, jc])
            # rmsnorm row: ln(mean(c^2) + eps) on scalar engine
            lnr = work.tile([1, LCH], F32R, name="lnr", tag="lnr", bufs=2)
            nc.scalar.activation(lnr, ssq, AF.Ln, bias=eps_col[0:1, :], scale=1.0 / LORA)
            return lnr

        def p0_rest(j, lnr):
            """q-projection m-chunks for l-chunk j + rsqrt-scale the latent."""
            jc = ds(j * LCH, LCH)
            # rsqrt broadcast: exp(-0.5 * ln(m)) across 128 partitions
            bct = pp.tile([128, LCH], F32, name="bc", tag="bc", bufs=1)
            mm(bct, mhalf_row, lnr, start=True, stop=True)
            for hi, dst in ((0, qn[0]), (1, qn[1])):
                acc = pp.tile([128, LCH], F32, name="acc", tag="accps", bufs=3)
                for k in range(NK):
                    mm(acc, w_sb[k][:, ds(MC_QN[hi], 128)], xt[j][k],
                       start=(k == 0), stop=(k == NK - 1))
                nc.vector.tensor_copy(dst[:, jc], acc)
                if hi == 0:
                    rs = work.tile([128, LCH], BF, name="rs", tag="rs", bufs=2)
                    nc.scalar.activation(rs, bct, AF.Exp)
            acc = pp.tile([128, LCH], F32, name="acc", tag="accps", bufs=3)
            for k in range(NK):
                mm(acc, w_sb[k][:, ds(MC_QR, 128)], xt[j][k],
                   start=(k == 0), stop=(k == NK - 1))
            nc.vector.tensor_copy(qr[:, jc], acc)
            for i in range(NLAT):
                nc.vector.tensor_mul(latT[i][:, jc], latT[i][:, jc], rs)

        def p2(j):
            """k^T and v tiles for this chunk's 4 s-tiles."""
            jc = ds(j * LCH, LCH)
            for si in range(4 * j, 4 * j + 4):
                pv = pp.tile([128, LCH], F32, name="pv", tag="accps", bufs=3)
                for i in range(NLAT):
                    mm(pv[:, 0 : HPC * VDIM], latT[i][:, ts(si, 128)], wu_sb[i],
                       start=(i == 0), stop=(i == NLAT - 1))
                nc.vector.tensor_copy(
                    vsb[:, ds(si * HPC * VDIM, HPC * VDIM)], pv[:, 0 : HPC * VDIM]
                )
            for h in range(HPC):
                pk = pp.tile([128, LCH], F32, name="pk", tag="accps", bufs=3)
                for i in range(NLAT):
                    mm(pk, we_sb[h][i], latT[i][:, jc],
                       start=(i == 0), stop=(i == NLAT - 1))
                nc.vector.tensor_copy(kT[h][:, jc], pk)

        def p3_scores(j, h):
            """Causal attention for (l-chunk j, head h) in transposed layout.

            Returns a closure that finishes the softmax normalization into
            outT[h] (invoke it after more TE work has been queued so the
            broadcast matmul doesn't head-of-line-block the tensor engine).
            """
            jc0 = j * LCH
            # groups: (si, col_lo, width, is_band); first must be full-width
            # (PSUM start), last must be full-width (PSUM stop carrier).
            groups = []
            if j > 0:
                for si in range(4 * j - 1):
                    groups.append((si, 0, LCH, False))
                for d in range(4):
                    groups.append((4 * j + d, 128 * d, LCH - 128 * d, True))
                groups.append((4 * j - 1, 0, LCH, False))
            else:
                groups.append((0, 0, LCH, True))
                groups.append((1, 128, 384, True))
                groups.append((2, 256, 256, True))
                groups.append((3, 0, LCH, True))  # carrier: full width, masked
            n = len(groups)
            ps_tiles = [None] * n
            es = [None] * n
            pcs_t = pp.tile([1, LCH], F32, name="pcs", tag="rowacc", bufs=2)
            po_t = pp.tile([128, LCH], F32, name="po", tag="po", bufs=2)

            def issue_ps(t):
                si, lo, w, band = groups[t]
                pst = pp.tile([128, LCH], F32, name="ps", tag="accps", bufs=3)
                mm(pst[:, 0:w], kT[h][:, ts(si, 128)], qn[h][:, ds(jc0 + lo, w)],
                   start=True, stop=False)
                mm(pst[:, 0:w], kpe[ds(64 * h, 64), ts(si, 128)],
                   qr[ds(64 * h, 64), ds(jc0 + lo, w)],
                   start=False, stop=not band)
                if band:
                    d = si - 4 * j
                    off = 384 - (128 * d - lo)
                    mm(pst[:, 0:w], negdiag, nbig_sb[:, ds(off, w)],
                       start=False, stop=True)
                ps_tiles[t] = pst

            def issue_exp(t):
                si, lo, w, band = groups[t]
                e = work.tile([128, LCH], BF, name="e", tag="e", bufs=3)
                nc.scalar.activation(e[:, 0:w], ps_tiles[t][:, 0:w], AF.Exp, scale=SCALE)
                es[t] = e

            def pcs_po(t):
                si, lo, w, band = groups[t]
                e = es[t]
                mm(pcs_t[0:1, ds(lo, w)], ones_col, e[:, 0:w],
                   start=(t == 0), stop=(t == n - 1))
                mm(po_t[:, ds(lo, w)], vsb[:, ds(si * HPC * VDIM + h * VDIM, VDIM)],
                   e[:, 0:w], start=(t == 0), stop=(t == n - 1))

            for t in range(n):
                issue_ps(t)
                issue_exp(t)
                if t >= 1:
                    pcs_po(t - 1)
            pcs_po(n - 1)
            lnr = work.tile([1, LCH], F32R, name="lnr2", tag="lnr", bufs=2)
            nc.scalar.activation(lnr, pcs_t, AF.Ln)

            def finish():
                bct = pp.tile([128, LCH], F32, name="bc2", tag="bc", bufs=1)
                mm(bct, mone_row, lnr, start=True, stop=True)
                rs = work.tile([128, LCH], BF, name="rs2", tag="rs", bufs=2)
                nc.scalar.activation(rs, bct, AF.Exp)
                nc.vector.tensor_mul(outT[h][:, ds(jc0, LCH)], po_t, rs)

            return finish

        def p4(j):
            """partial o_proj for this chunk's 4 l-tiles -> y DMA (sync q)."""
            for i in range(4 * j, 4 * j + 4):
                for nn in range(NJ):
                    py = pp.tile([128, LCH], F32, name="py", tag="accps", bufs=3)
                    mm(py, outT[0][:, ts(i, 128)], wo_sb[0][:, ds(nn * LCH, LCH)],
                       start=True, stop=False)
                    mm(py, outT[1][:, ts(i, 128)], wo_sb[1][:, ds(nn * LCH, LCH)],
                       start=False, stop=True)
                    ysb = work.tile([128, LCH], BF, name="ysb", tag="ysb", bufs=3)
                    nc.vector.tensor_copy(ysb, py)
                    nc.sync.dma_start(out=y_d[ts(i, 128), ds(nn * LCH, LCH)], in_=ysb)

        # ---- main pipeline ----
        lnr0 = p0_lat(0)
        p0_rest(0, lnr0)
        for j in range(NJ):
            p2(j)
            if j + 1 < NJ:
                load_xt(j + 1)
            fin0 = p3_scores(j, 0)
            fin1 = p3_scores(j, 1)
            fin0()
            if j + 1 < NJ:
                lnr = p0_lat(j + 1)
                fin1()
                p4(j)
                p0_rest(j + 1, lnr)
            else:
                fin1()
                p4(j)

    _split_excess_waits(nc)
    return nc


_NC_CACHE = None


def _get_nc():
    global _NC_CACHE
    if _NC_CACHE is None:
        _NC_CACHE = _build_nc()
    return _NC_CACHE


def _make_in_maps(x, Wq, Wkv_a, kv_ln_w, W_embed, W_unembed, Wo):
    bf16 = ml_dtypes.bfloat16
    xT = np.ascontiguousarray(np.asarray(x, dtype=np.float32)[0].T).astype(bf16)
    Wq = np.asarray(Wq, dtype=np.float32)
    Wkv_a = np.asarray(Wkv_a, dtype=np.float32)
    kv_ln_w = np.asarray(kv_ln_w, dtype=np.float32)
    W_embed = np.asarray(W_embed, dtype=np.float32)
    W_unembed = np.asarray(W_unembed, dtype=np.float32)
    Wo = np.asarray(Wo, dtype=np.float32)

    Wq3 = Wq.reshape(HID, H, QDIM)
    # invalid-mask band template: nbig[p, q] = 1 iff (q - 384) < p
    q_idx = np.arange(896) - 384
    p_idx = np.arange(128)
    nbig = (q_idx[None, :] < p_idx[:, None]).astype(bf16)
    negdiag = (NEGBIG * np.eye(128, dtype=np.float32)).astype(bf16)

    in_maps = []
    for c in range(NCORES):
        h0, h1 = HPC * c, HPC * c + 1
        # columns: lat(512) | kpe(64) | qn h0 (128) | qn h1 (128) | qr (h0 64 | h1 64)
        wqkv = np.concatenate(
            [
                Wkv_a[:, :LORA],
                Wkv_a[:, LORA:],
                Wq3[:, h0, :NOPE],
                Wq3[:, h1, :NOPE],
                Wq3[:, h0, NOPE:],
                Wq3[:, h1, NOPE:],
            ],
            axis=1,
        )
        we = np.ascontiguousarray(W_embed[[h0, h1]] * kv_ln_w[None, :, None])
        wu = np.ascontiguousarray(
            np.concatenate([W_unembed[h0].T, W_unembed[h1].T], axis=1) * kv_ln_w[:, None]
        )
        in_maps.append(
            {
                "xT": xT,
                "wqkv": np.ascontiguousarray(wqkv).astype(bf16),
                "we": we.astype(bf16),
                "wu": wu.astype(bf16),
                "wo": np.ascontiguousarray(
                    Wo[h0 * VDIM : (h1 + 1) * VDIM]
                ).astype(bf16),
                "nbig": nbig,
                "negdiag": negdiag,
                "ones_col_d": np.ones((128, 1), np.float32).astype(bf16),
                "mhalf_row_d": np.full((1, 128), -0.5, np.float32),
                "mone_row_d": np.full((1, 128), -1.0, np.float32),
            }
        )
    return in_maps


def run(trace=False, tmpdir=None, **inputs):
    """Run the SPMD kernel; returns (full_output, BassKernelResults)."""
    inputs.pop("mask", None)  # causal structure is hardcoded
    nc = _get_nc()
    in_maps = _make_in_maps(**inputs)
    res = run_bass_kernel_spmd(
        nc, in_maps, core_ids=list(range(NCORES)), trace=trace, tmpdir=tmpdir
    )
    y = np.zeros((L, HID), dtype=np.float32)
    for c in range(NCORES):
        y += np.asarray(res.results[c]["y"], dtype=np.float32)
    return y.reshape(B, L, HID), res


def kernel(**inputs):
    y, _ = run(trace=False, **inputs)
    return y


# revision 8
# speedup vs baseline: 1.0519x; 1.0519x over previous
"""Bass/Trainium2 kernel for Kimi-style MLA attention (nn_KimiMLAAttention).

Strategy (8 NeuronCores, tensor-parallel over heads):
  - 16 heads -> 2 heads per core. Each core computes q-projection for its 2
    heads, the (replicated) compressed-kv projection + rmsnorm, per-head
    k-embed / v-unembed from the shared latent, causal attention in a
    TRANSPOSED score layout (scores^T[s, l]), and a partial o_proj against
    its 2-head slice of Wo. Host sums the 8 partial outputs.

Perf design (v2):
  - All SBUF operands bf16 (PE streams bf16 at 1 col/cycle like fp32r, but
    DMA/SBUF/vector costs halve); PSUM accumulation stays fp32.
  - Everything is pipelined per 512-column l-chunk: P0 projections ->
    rmsnorm -> P2 k/v embed -> P3 attention (2 heads) -> P4 partial o_proj,
    with the next chunk's projection matmuls issued between dependency
    stalls so the tensor engine never idles (PE clock ramps to 2.4 GHz only
    after ~3us of continuous execution; gaps drop it to 1.2 GHz).
  - P0 runs k-innermost into 2 rotating PSUM banks (no 8-bank barrier).
  - Causal masking is done ON the PE: a -BIG * lower-triangle matmul is
    accumulated into the score PSUM tile before exp, so the scalar exp
    output needs no vector-side mask multiply. Diagonal-band tiles stream
    only their valid column suffix (~half the band work skipped).
  - Normalizations (rmsnorm rsqrt, softmax 1/colsum) avoid the slow DVE
    InstReciprocal: row = Ln(sum) on scalar -> rank-1 broadcast matmul with
    a -1 (or -0.5) row -> Exp on scalar gives exp(-ln x) = 1/x (or x^-1/2)
    broadcast across partitions.
  - Score matmuls are issued one s-tile ahead of the exp-dependent
    colsum/AV matmuls (software pipelining, PSUM tag ring buffers).
"""

from contextlib import ExitStack

import numpy as np
import ml_dtypes

import concourse.bass as bass
import concourse.tile as tile
from concourse import mybir
from concourse.bass import ds, ts
from concourse.bass_utils import run_bass_kernel_spmd

F32 = mybir.dt.float32
F32R = mybir.dt.float32r
BF = mybir.dt.bfloat16
AF = mybir.ActivationFunctionType


def _patch_tile_tail_drain():
    """walrus's CoreV3 codegen rejects the TileContext tail drain when it
    carries >1 sem waits ("Too many sync wait commands"). Split the waits
    across multiple single-wait drain instructions on the sync engine."""
    if getattr(tile.TileContext, "_tail_drain_patched", False):
        return
    from concourse.vector_clock import ScopedClock

    def _drain_and_barrier(self, tick_clock, wait_clock):
        nc = self.nc
        drain_inst = nc.sync.drain()
        wait_clock.add_sem_waits(
            drain_inst.ins, ScopedClock({None: tick_clock.global_clock})
        )
        inst = drain_inst.ins
        si = inst.sync_info
        if si is not None and si.on_wait is not None and len(si.on_wait) > 1:
            waits = list(si.on_wait)
            upd = list(si.on_update) if si.on_update else []
            inst.sync_info = mybir.SyncInfo(on_wait=waits[:1], on_update=[])
            for i, w in enumerate(waits[1:]):
                extra = nc.sync.drain()
                last = i == len(waits) - 2
                extra.ins.sync_info = mybir.SyncInfo(
                    on_wait=[w], on_update=upd if last else []
                )
        nc.all_engine_barrier()
        assert self.sems is not None
        popped = nc._tile_sem_poison_stack.pop()
        assert popped is self._sem_poison
        nc.clear_and_free_semaphores(list(self.sems.allocated().values()))
        nc.all_engine_barrier()

    tile.TileContext._drain_and_barrier = _drain_and_barrier
    tile.TileContext._tail_drain_patched = True


_patch_tile_tail_drain()


def _split_excess_waits(nc, max_waits=1):
    """walrus's per-instruction sync-wait slots are tiny on this compiler
    build; hoist excess sem waits onto same-engine NoOp carriers placed
    immediately before the instruction (waits fire earlier in the same
    engine stream, so ordering semantics are preserved)."""
    for f in nc.m.functions:
        for bb in f.blocks:
            insts = bb.instructions
            if not any(
                i.sync_info is not None
                and i.sync_info.on_wait
                and len(i.sync_info.on_wait) > max_waits
                for i in insts
            ):
                continue
            out = []
            for inst in insts:
                si = inst.sync_info
                if si is not None and si.on_wait and len(si.on_wait) > max_waits:
                    waits = list(si.on_wait)
                    for w in waits[:-max_waits]:
                        nop = mybir.InstNoOp(
                            name=nc.get_next_instruction_name(), ins=[], outs=[]
                        )
                        nop.engine = inst.engine
                        nop.sync_info = mybir.SyncInfo(on_wait=[w], on_update=[])
                        out.append(nop)
                    inst.sync_info = mybir.SyncInfo(
                        on_wait=waits[-max_waits:],
                        on_update=list(si.on_update) if si.on_update else [],
                    )
                out.append(inst)
            bb.instructions = out


B, L, HID = 1, 2048, 2048
H = 16
NOPE, ROPE, VDIM, LORA = 128, 64, 128, 512
QDIM = NOPE + ROPE
EPS = 1e-5
SCALE = QDIM**-0.5
NCORES = 8
HPC = H // NCORES  # 2 heads per core

LCH = 512  # moving-operand chunk (max fp32 N per matmul / PSUM bank)
NJ = L // LCH  # 4 l-chunks
NK = HID // 128  # 16 contraction tiles for projections
NS = L // 128  # 16 s(key)-tiles
NLAT = LORA // 128  # 4 latent partition tiles
WCOLS = 960  # fused projection weight columns
NEGBIG = -1000.0  # pre-exp causal mask bias (NEGBIG*SCALE ~ -72 per unit)

# wqkv column layout (host packs in this order):
#   lat0 lat1 lat2 lat3 (4x128) | kpe (64) | qn0 (128) | qn1 (128) | qr (128)
MC_LAT = [0, 128, 256, 384]
MC_KPE = 512
MC_QN = [576, 704]
MC_QR = 832


def _build_nc():
    nc = bass.Bass()
    xT_d = nc.dram_tensor("xT", [HID, L], BF, kind="ExternalInput")
    wqkv_d = nc.dram_tensor("wqkv", [HID, WCOLS], BF, kind="ExternalInput")
    we_d = nc.dram_tensor("we", [HPC, LORA, NOPE], BF, kind="ExternalInput")
    wu_d = nc.dram_tensor("wu", [LORA, HPC * VDIM], BF, kind="ExternalInput")
    wo_d = nc.dram_tensor("wo", [HPC * VDIM, HID], BF, kind="ExternalInput")
    nbig_d = nc.dram_tensor("nbig", [128, 896], BF, kind="ExternalInput")
    negdiag_d = nc.dram_tensor("negdiag", [128, 128], BF, kind="ExternalInput")
    ones_col_d = nc.dram_tensor("ones_col_d", [128, 1], BF, kind="ExternalInput")
    mhalf_row_d = nc.dram_tensor("mhalf_row_d", [1, 128], F32R, kind="ExternalInput")
    mone_row_d = nc.dram_tensor("mone_row_d", [1, 128], F32R, kind="ExternalInput")
    y_d = nc.dram_tensor("y", [L, HID], BF, kind="ExternalOutput")

    mm = nc.tensor.matmul

    with tile.TileContext(nc) as tc, ExitStack() as ctx:
        persist = ctx.enter_context(tc.tile_pool(name="persist", bufs=1))
        xtp = ctx.enter_context(tc.tile_pool(name="xtp", bufs=2))
        work = ctx.enter_context(tc.tile_pool(name="work", bufs=1))
        pp = ctx.enter_context(tc.tile_pool(name="pp", bufs=1, space="PSUM"))

        # ---- persistent SBUF tiles ----
        qn = [persist.tile([128, L], BF, name=f"qn{h}", tag=f"qn{h}") for h in range(HPC)]
        qr = persist.tile([128, L], BF, name="qr", tag="qr")
        kpe = persist.tile([128, L], BF, name="kpe", tag="kpe")
        latT = [persist.tile([128, L], BF, name=f"latT{i}", tag=f"latT{i}") for i in range(NLAT)]
        kT = [persist.tile([128, L], BF, name=f"kT{h}", tag=f"kT{h}") for h in range(HPC)]
        vsb = persist.tile([128, NS * HPC * VDIM], BF, name="vsb", tag="vsb")
        outT = [persist.tile([128, L], BF, name=f"outT{h}", tag=f"outT{h}") for h in range(HPC)]
        nbig_sb = persist.tile([128, 896], BF, name="nbig_sb", tag="nbig_sb")
        negdiag = persist.tile([128, 128], BF, name="negdiag", tag="negdiag")
        ones_col = persist.tile([128, 1], BF, name="ones_col", tag="ones_col")
        mhalf_row = persist.tile([1, 128], F32R, name="mhalf_row", tag="mhalf_row")
        mone_row = persist.tile([1, 128], F32R, name="mone_row", tag="mone_row")
        w_sb = [persist.tile([128, WCOLS], BF, name=f"w{k}", tag=f"w{k}") for k in range(NK)]
        we_sb = [
            persist.tile([128, NLAT * NOPE], BF, name=f"we{h}", tag=f"we{h}")
            for h in range(HPC)
        ]
        wu_sb = persist.tile([128, NLAT * HPC * VDIM], BF, name="wu", tag="wu")
        wo_sb = [persist.tile([128, HID], BF, name=f"wo{h}", tag=f"wo{h}") for h in range(HPC)]
        eps_col = persist.tile([128, 1], F32, name="eps_col", tag="eps_col")
        nc.vector.memset(eps_col, EPS)

        # ---- startup DMAs ----
        # sync queue: fused-projection weights first (TE consumes k-order),
        # then the small attention constants, then o_proj weights.
        for k in range(NK):
            nc.sync.dma_start(out=w_sb[k], in_=wqkv_d[ts(k, 128), :])
        nc.sync.dma_start(out=ones_col, in_=ones_col_d[:, :])
        nc.sync.dma_start(out=mhalf_row, in_=mhalf_row_d[:, :])
        nc.sync.dma_start(out=mone_row, in_=mone_row_d[:, :])
        nc.sync.dma_start(out=negdiag, in_=negdiag_d[:, :])
        nc.sync.dma_start(out=nbig_sb, in_=nbig_d[:, :])
        for h in range(HPC):
            nc.sync.dma_start(out=wo_sb[h], in_=wo_d[ts(h, 128), :])
        # activation queue: x tiles for chunk 0, then embed/unembed weights.
        xtb = [None] * NJ

        def load_xt(j):
            t = xtp.tile([128, NK * LCH], BF, name="xtb", tag="xtb", bufs=2)
            nc.scalar.dma_start(
                out=t,
                in_=xT_d[:, ds(j * LCH, LCH)].rearrange("(k p) l -> p k l", p=128),
            )
            xtb[j] = t

        def xt(j, k):
            return xtb[j][:, ds(k * LCH, LCH)]

        load_xt(0)
        for h in range(HPC):
            nc.scalar.dma_start(
                out=we_sb[h],
                in_=we_d[h].rearrange("(i p) n -> p i n", p=128),
            )
        nc.scalar.dma_start(
            out=wu_sb, in_=wu_d[:, :].rearrange("(i p) v -> p i v", p=128)
        )

        # ---- phase bodies ----
        def p0_lat(j):
            """latent+kpe m-chunks for l-chunk j, squares+ssq+Ln of rmsnorm."""
            jc = ds(j * LCH, LCH)
            ssq = pp.tile([1, LCH], F32, name="ssq", tag="rowacc", bufs=2)
            for i in range(NLAT):
                acc = pp.tile([128, LCH], F32, name="acc", tag="accps", bufs=3)
                for k in range(NK):
                    mm(acc, w_sb[k][:, ds(MC_LAT[i], 128)], xt(j, k),
                       start=(k == 0), stop=(k == NK - 1))
                nc.vector.tensor_copy(latT[i][:, jc], acc)
                sq = work.tile([128, LCH], BF, name="sq", tag="sq", bufs=2)
                nc.scalar.activation(sq, acc, AF.Square)
                mm(ssq, ones_col, sq, start=(i == 0), stop=(i == NLAT - 1))
            # kpe chunk (64 cols), duplicated onto partitions 64:128 via DMA
            acc = pp.tile([128, LCH], F32, name="acc", tag="accps", bufs=3)
            for k in range(NK):
                mm(acc[0:64, :], w_sb[k][:, ds(MC_KPE, 64)], xt(j, k),
                   start=(k == 0), stop=(k == NK - 1))
            nc.vector.tensor_copy(kpe[0:64, jc], acc[0:64, :])
            nc.sync.dma_start(out=kpe[64:128, jc], in_=kpe[0:64, jc])
            # rmsnorm row: ln(mean(c^2) + eps) on scalar engine
            lnr = work.tile([1, LCH], F32R, name="lnr", tag="lnr", bufs=2)
            nc.scalar.activation(lnr, ssq, AF.Ln, bias=eps_col[0:1, :], scale=1.0 / LORA)
            return lnr

        def p0_rest(j, lnr):
            """q-projection m-chunks for l-chunk j + rsqrt-scale the latent."""
            jc = ds(j * LCH, LCH)
            # rsqrt broadcast: exp(-0.5 * ln(m)) across 128 partitions
            bct = pp.tile([128, LCH], F32, name="bc", tag="bc", bufs=1)
            mm(bct, mhalf_row, lnr, start=True, stop=True)
            for hi, dst in ((0, qn[0]), (1, qn[1])):
                acc = pp.tile([128, LCH], F32, name="acc", tag="accps", bufs=3)
                for k in range(NK):
                    mm(acc, w_sb[k][:, ds(MC_QN[hi], 128)], xt(j, k),
                       start=(k == 0), stop=(k == NK - 1))
                nc.vector.tensor_copy(dst[:, jc], acc)
                if hi == 0:
                    rs = work.tile([128, LCH], BF, name="rs", tag="rs", bufs=2)
                    nc.scalar.activation(rs, bct, AF.Exp)
            acc = pp.tile([128, LCH], F32, name="acc", tag="accps", bufs=3)
            for k in range(NK):
                mm(acc, w_sb[k][:, ds(MC_QR, 128)], xt(j, k),
                   start=(k == 0), stop=(k == NK - 1))
            nc.vector.tensor_copy(qr[:, jc], acc)
            for i in range(NLAT):
                nc.vector.tensor_mul(latT[i][:, jc], latT[i][:, jc], rs)

        def p2(j):
            """k^T and v tiles for this chunk's 4 s-tiles."""
            jc = ds(j * LCH, LCH)
            for si in range(4 * j, 4 * j + 4):
                pv = pp.tile([128, LCH], F32, name="pv", tag="accps", bufs=3)
                for i in range(NLAT):
                    mm(pv[:, 0 : HPC * VDIM], latT[i][:, ts(si, 128)],
                       wu_sb[:, ds(i * HPC * VDIM, HPC * VDIM)],
                       start=(i == 0), stop=(i == NLAT - 1))
                nc.vector.tensor_copy(
                    vsb[:, ds(si * HPC * VDIM, HPC * VDIM)], pv[:, 0 : HPC * VDIM]
                )
            for h in range(HPC):
                pk = pp.tile([128, LCH], F32, name="pk", tag="accps", bufs=3)
                for i in range(NLAT):
                    mm(pk, we_sb[h][:, ds(i * NOPE, NOPE)], latT[i][:, jc],
                       start=(i == 0), stop=(i == NLAT - 1))
                nc.vector.tensor_copy(kT[h][:, jc], pk)

        def p3_scores(j, h):
            """Causal attention for (l-chunk j, head h) in transposed layout.

            Returns a closure that finishes the softmax normalization into
            outT[h] (invoke it after more TE work has been queued so the
            broadcast matmul doesn't head-of-line-block the tensor engine).
            """
            jc0 = j * LCH
            # groups: (si, col_lo, width, is_band); first must be full-width
            # (PSUM start), last must be full-width (PSUM stop carrier).
            groups = []
            if j > 0:
                for si in range(4 * j - 1):
                    groups.append((si, 0, LCH, False))
                for d in range(4):
                    groups.append((4 * j + d, 128 * d, LCH - 128 * d, True))
                groups.append((4 * j - 1, 0, LCH, False))
            else:
                groups.append((0, 0, LCH, True))
                groups.append((1, 128, 384, True))
                groups.append((2, 256, 256, True))
                groups.append((3, 0, LCH, True))  # carrier: full width, masked
            n = len(groups)
            ps_tiles = [None] * n
            es = [None] * n
            pcs_t = pp.tile([1, LCH], F32, name="pcs", tag="rowacc", bufs=2)
            po_t = pp.tile([128, LCH], F32, name="po", tag="po", bufs=2)

            def issue_ps(t):
                si, lo, w, band = groups[t]
                pst = pp.tile([128, LCH], F32, name="ps", tag="accps", bufs=3)
                mm(pst[:, 0:w], kT[h][:, ts(si, 128)], qn[h][:, ds(jc0 + lo, w)],
                   start=True, stop=False)
                mm(pst[:, 0:w], kpe[ds(64 * h, 64), ts(si, 128)],
                   qr[ds(64 * h, 64), ds(jc0 + lo, w)],
                   start=False, stop=not band)
                if band:
                    d = si - 4 * j
                    off = 384 - (128 * d - lo)
                    mm(pst[:, 0:w], negdiag, nbig_sb[:, ds(off, w)],
                       start=False, stop=True)
                ps_tiles[t] = pst

            def issue_exp(t):
                si, lo, w, band = groups[t]
                e = work.tile([128, LCH], BF, name="e", tag="e", bufs=4)
                nc.scalar.activation(e[:, 0:w], ps_tiles[t][:, 0:w], AF.Exp, scale=SCALE)
                es[t] = e

            def pcs_po(t):
                si, lo, w, band = groups[t]
                e = es[t]
                mm(pcs_t[0:1, ds(lo, w)], ones_col, e[:, 0:w],
                   start=(t == 0), stop=(t == n - 1))
                mm(po_t[:, ds(lo, w)], vsb[:, ds(si * HPC * VDIM + h * VDIM, VDIM)],
                   e[:, 0:w], start=(t == 0), stop=(t == n - 1))

            issue_ps(0)
            issue_exp(0)
            if n > 1:
                issue_ps(1)
                issue_exp(1)
            for t in range(2, n):
                issue_ps(t)
                issue_exp(t)
                pcs_po(t - 2)
            if n > 1:
                pcs_po(n - 2)
            pcs_po(n - 1)
            lnr = work.tile([1, LCH], F32R, name="lnr2", tag="lnr", bufs=2)
            nc.scalar.activation(lnr, pcs_t, AF.Ln)

            def finish():
                bct = pp.tile([128, LCH], F32, name="bc2", tag="bc", bufs=1)
                mm(bct, mone_row, lnr, start=True, stop=True)
                rs = work.tile([128, LCH], BF, name="rs2", tag="rs", bufs=2)
                nc.scalar.activation(rs, bct, AF.Exp)
                nc.vector.tensor_mul(outT[h][:, ds(jc0, LCH)], po_t, rs)

            return finish

        def p4(j):
            """partial o_proj for this chunk's 4 l-tiles -> y DMA (sync q)."""
            for i in range(4 * j, 4 * j + 4):
                ysb = work.tile([128, HID], BF, name="ysb", tag="ysb", bufs=2)
                for nn in range(NJ):
                    py = pp.tile([128, LCH], F32, name="py", tag="accps", bufs=3)
                    mm(py, outT[0][:, ts(i, 128)], wo_sb[0][:, ds(nn * LCH, LCH)],
                       start=True, stop=False)
                    mm(py, outT[1][:, ts(i, 128)], wo_sb[1][:, ds(nn * LCH, LCH)],
                       start=False, stop=True)
                    nc.scalar.copy(ysb[:, ds(nn * LCH, LCH)], py)
                nc.sync.dma_start(out=y_d[ts(i, 128), :], in_=ysb)

        # ---- main pipeline ----
        lnr0 = p0_lat(0)
        p0_rest(0, lnr0)
        for j in range(NJ):
            p2(j)
            if j + 1 < NJ:
                load_xt(j + 1)
            fin0 = p3_scores(j, 0)
            fin1 = p3_scores(j, 1)
            fin0()
            if j + 1 < NJ:
                lnr = p0_lat(j + 1)
                fin1()
                p0_rest(j + 1, lnr)
                p4(j)
            else:
                fin1()
                p4(j)

    _split_excess_waits(nc)
    return nc


_NC_CACHE = None


def _get_nc():
    global _NC_CACHE
    if _NC_CACHE is None:
        _NC_CACHE = _build_nc()
    return _NC_CACHE


def _make_in_maps(x, Wq, Wkv_a, kv_ln_w, W_embed, W_unembed, Wo):
    bf16 = ml_dtypes.bfloat16
    xT = np.ascontiguousarray(np.asarray(x, dtype=np.float32)[0].T).astype(bf16)
    Wq = np.asarray(Wq, dtype=np.float32)
    Wkv_a = np.asarray(Wkv_a, dtype=np.float32)
    kv_ln_w = np.asarray(kv_ln_w, dtype=np.float32)
    W_embed = np.asarray(W_embed, dtype=np.float32)
    W_unembed = np.asarray(W_unembed, dtype=np.float32)
    Wo = np.asarray(Wo, dtype=np.float32)

    Wq3 = Wq.reshape(HID, H, QDIM)
    # invalid-mask band template: nbig[p, q] = 1 iff (q - 384) < p
    q_idx = np.arange(896) - 384
    p_idx = np.arange(128)
    nbig = (q_idx[None, :] < p_idx[:, None]).astype(bf16)
    negdiag = (NEGBIG * np.eye(128, dtype=np.float32)).astype(bf16)

    in_maps = []
    for c in range(NCORES):
        h0, h1 = HPC * c, HPC * c + 1
        # columns: lat(512) | kpe(64) | qn h0 (128) | qn h1 (128) | qr (h0 64 | h1 64)
        wqkv = np.concatenate(
            [
                Wkv_a[:, :LORA],
                Wkv_a[:, LORA:],
                Wq3[:, h0, :NOPE],
                Wq3[:, h1, :NOPE],
                Wq3[:, h0, NOPE:],
                Wq3[:, h1, NOPE:],
            ],
            axis=1,
        )
        we = np.ascontiguousarray(W_embed[[h0, h1]] * kv_ln_w[None, :, None])
        wu = np.ascontiguousarray(
            np.concatenate([W_unembed[h0].T, W_unembed[h1].T], axis=1) * kv_ln_w[:, None]
        )
        in_maps.append(
            {
                "xT": xT,
                "wqkv": np.ascontiguousarray(wqkv).astype(bf16),
                "we": we.astype(bf16),
                "wu": wu.astype(bf16),
                "wo": np.ascontiguousarray(
                    Wo[h0 * VDIM : (h1 + 1) * VDIM]
                ).astype(bf16),
                "nbig": nbig,
                "negdiag": negdiag,
                "ones_col_d": np.ones((128, 1), np.float32).astype(bf16),
                "mhalf_row_d": np.full((1, 128), -0.5, np.float32),
                "mone_row_d": np.full((1, 128), -1.0, np.float32),
            }
        )
    return in_maps


def run(trace=False, tmpdir=None, **inputs):
    """Run the SPMD kernel; returns (full_output, BassKernelResults)."""
    inputs.pop("mask", None)  # causal structure is hardcoded
    nc = _get_nc()
    in_maps = _make_in_maps(**inputs)
    res = run_bass_kernel_spmd(
        nc, in_maps, core_ids=list(range(NCORES)), trace=trace, tmpdir=tmpdir
    )
    y = np.zeros((L, HID), dtype=np.float32)
    for c in range(NCORES):
        y += np.asarray(res.results[c]["y"], dtype=np.float32)
    return y.reshape(B, L, HID), res


def kernel(**inputs):
    y, _ = run(trace=False, **inputs)
    return y
